# revision 12
# baseline (speedup 1.0000x reference)
"""Trainium2 Bass kernel for EnhancedMambaStateSpace.

Full inputs in, full output out. Data-parallel over batch across 8 cores
(2 batch rows per core); SSM params replicated and pre-folded on host.

Math (per batch row b):
  xc = depthwise_conv1d(x, conv_w, pad=1) + conv_b
  sel = softplus(xc @ sel_W.T + sel_b + selection_bias)
  delta = softplus(xc @ delta_W.T + delta_b)
  A = -exp(A_log); Ad = exp(delta * A)
  Bx = (Ad - 1)/(A + 1e-8) * sel * (xc @ Bm.T)
  s_t = Ad_t * s_{t-1} + Bx_t  (scan over L, keep last)
  y = s_L @ Cm.T + xc[:, -1] @ Dm.T

Device layout: x is transposed and CHUNKED on host into
[P, NCH, 2h, 2b, CW] (CW = CH+3: each chunk carries its 3-col conv halo),
in both fp16 and fp8e4m3 — each chunk lands with ONE contiguous-per-
partition DMA (128 descriptor rows), keeping HW descriptor-gen off the
critical path. The conv is fully folded into the projection weights
(3 taps = 3 shifted matmul streams), so no explicit conv runs on any
engine. The sel|delta block runs in fp8 with DoubleRow perf mode: one
pass contracts both 128-halves of D (h is the DoubleRow k-tile dim); the
fp8 weights are pre-scaled by S8 on host and descaled for free via the
exp ACT's scale operand. The Bm block (numerically critical: its output
feeds y linearly) stays fp16. The recurrence is a DVE tensor_tensor_scan,
batch-packed [b0|b1] on 128 partitions. Warm-up dummy matmuls (discarded
via start=True PSUM reset) bring the PE out of its low p-state before
real data lands. The last chunk's elementwise tail runs in two
half-chunks to shorten the end-of-kernel serial chain.
"""

from contextlib import ExitStack

import ml_dtypes
import numpy as np

import concourse.bacc as bacc
import concourse.tile as tile
from concourse import mybir
from concourse.bass_utils import run_bass_kernel_spmd

B, L, D, N, O = 16, 4096, 256, 64, 256
P = 128          # partitions
CH = 1024        # tokens per chunk
NCH = L // CH    # 4 chunks
CW = CH + 3      # chunk window incl 3-col conv halo
BPC = 2          # batch rows per core
NCORES = 8
LW = L + 2       # pad col 0 (x[-1]=0) and col L+1 (x[L]=0)
FM = 512         # ISA max moving free dim
S8 = 256.0       # fp8 weight pre-scale (descaled in the exp ACT)
NWARM = 6        # PE p-state warm-up matmuls

FP = mybir.dt.float32
XDT = mybir.dt.float16
F8 = mybir.dt.float8e4
AOP = mybir.AluOpType
DR = mybir.MatmulPerfMode.DoubleRow

_ONE_TABLE = "natural_log_exp_and_others"


def _patch_act_tables():
    """Keep Exp/Ln/Copy resolvable only via one ACT table so the
    act-table-load pass never thrashes between tables (1283ns per load)."""
    import concourse.hw_specs as hw_specs
    import concourse.bacc as _bacc
    if getattr(_bacc, "_act_tables_patched", False):
        return
    orig = hw_specs.get_activation_tables

    def patched(module_arch):
        tabs = orig(module_arch)
        drop = {mybir.ActivationFunctionType.Exp,
                mybir.ActivationFunctionType.Ln,
                mybir.ActivationFunctionType.Copy}
        out = {}
        for name, funcs in tabs.items():
            if name == _ONE_TABLE:
                out[name] = funcs
            else:
                out[name] = funcs - drop
        return out

    _bacc.get_activation_tables = patched
    _bacc._act_tables_patched = True


def _build_program():
    _patch_act_tables()
    nc = bacc.Bacc("TRN2", target_bir_lowering=False, debug=False)

    # x chunked on host: xs[kd, c, h, b, w]; w = global col (1+t) - c*CH
    xs = nc.dram_tensor("xs", [P, NCH, 2, BPC, CW], XDT,
                        kind="ExternalInput").ap()
    xs8 = nc.dram_tensor("xs8", [P, NCH, 2, BPC, CW], F8,
                         kind="ExternalInput").ap()
    # fp8 tap-folded lhsT for sel|delta: wk8[kd, h, k, j] =
    #   S8 * Wsd[j, 128h+kd] * cw[128h+kd, k]; h is the DoubleRow k-tile dim
    wk8 = nc.dram_tensor("wk8", [P, 2, 3, P], F8, kind="ExternalInput").ap()
    # fp16 tap-folded lhsT for Bm: wkB[kd, h, k, n] = Bm[n, 128h+kd]*cw[.., k]
    wkB = nc.dram_tensor("wkB", [P, 2, 3, N], XDT, kind="ExternalInput").ap()
    # col 0: softplus bias (sel|delta), col 1: A tiled, col 2: Bm@conv_b tiled
    pcols = nc.dram_tensor("pcols", [P, 3], FP, kind="ExternalInput").ap()
    cmT = nc.dram_tensor("cmT", [P, 2 * O], XDT, kind="ExternalInput").ap()
    dmT = nc.dram_tensor("dmT", [P, 2, 2, O], XDT, kind="ExternalInput").ap()
    ybias = nc.dram_tensor("ybias", [1, 2 * O], FP, kind="ExternalInput").ap()
    y = nc.dram_tensor("y", [1, 2 * O], FP, kind="ExternalOutput").ap()

    with tile.TileContext(nc) as tc, ExitStack() as ctx:
        consts = ctx.enter_context(tc.tile_pool(name="consts", bufs=1))
        xtp = ctx.enter_context(tc.tile_pool(name="xtp", bufs=1))
        nsb = ctx.enter_context(tc.tile_pool(name="nsb", bufs=2))
        scanp = ctx.enter_context(tc.tile_pool(name="scanp", bufs=2))
        psum = ctx.enter_context(tc.tile_pool(name="psum", bufs=1, space="PSUM"))

        wk8_sb = consts.tile([P, 2, 3, P], F8, tag="wk8")
        # never-written-by-DMA garbage weights for the p-state warm-up
        dumw_sb = consts.tile([P, 2, P], F8, tag="dumw")
        wkB_sb = consts.tile([P, 2, 3, N], XDT, tag="wkB")
        pcols_sb = consts.tile([P, 3], FP, tag="pcols")
        cmT_sb = consts.tile([P, 2 * O], XDT, tag="cmT")
        dmT_sb = consts.tile([P, 2, 2, O], XDT, tag="dmT")
        ybias_sb = consts.tile([1, 2 * O], FP, tag="ybias")

        # whole-sequence chunked x, SBUF-resident, fp16 (Bm block) and
        # fp8 (sel|delta block); chunk-major so each chunk's DMA is one
        # contiguous run per partition
        xt = xtp.tile([P, NCH, 2, BPC, CW], XDT, tag="xt")
        xt8 = xtp.tile([P, NCH, 2, BPC, CW], F8, tag="xt8")

        # warm-up weights and rhs region zeroed so the warm-up matmuls
        # contribute exact zeros to the psd c0 accumulation group (and
        # can't read NaN-pattern garbage); gpsimd is otherwise idle
        nc.gpsimd.memset(dumw_sb, 0.0)
        nc.gpsimd.memset(xt8[:, NCH - 1, :, 1, 0:FM + NWARM + 1], 0.0)

        # sync carries the fp8 stream (psd block inputs), scalar carries
        # the fp16 stream (pP block inputs) + tail consts, so the psd
        # chain's inputs land first on the early-slow DMA path.
        nc.sync.dma_start(out=wk8_sb, in_=wk8)
        nc.sync.dma_start(out=xt8[:, 0], in_=xs8[:, 0])
        nc.sync.dma_start(out=pcols_sb, in_=pcols)
        nc.scalar.dma_start(out=wkB_sb, in_=wkB)
        nc.scalar.dma_start(out=xt[:, 0], in_=xs[:, 0])
        for c in range(1, NCH):
            nc.sync.dma_start(out=xt8[:, c], in_=xs8[:, c])
            nc.scalar.dma_start(out=xt[:, c], in_=xs[:, c])
        nc.scalar.dma_start(out=cmT_sb, in_=cmT)
        nc.scalar.dma_start(out=dmT_sb, in_=dmT)
        nc.scalar.dma_start(out=ybias_sb, in_=ybias)

        def emit_mm_block(c, psd, pP, warm=False):
            """Projection matmuls for chunk c: psd (fp8 DoubleRow, both
            halves contracted per pass) first to unblock the softplus
            chain, then the fp16 pP block. f-ordered so the f0 half only
            needs... both f come from the same chunk DMA now."""
            if warm:
                # p-state warm-up: zero-weight matmuls accumulating exact
                # zeros into psd (b0, f0)'s group so the PE exits its low
                # p-state before real data lands. rhs reads the zeroed
                # head of xt8's last chunk (DMA'd much later), so the
                # only input dep is the gpsimd memset.
                with tc.high_priority():
                    for i in range(NWARM):
                        nc.tensor.matmul(
                            out=psd[:, 0, 0:FM],
                            lhsT=dumw_sb,
                            rhs=xt8[:, NCH - 1, :, 1, 1 + i:1 + i + FM],
                            start=(i == 0), stop=False,
                            perf_mode=DR, skip_group_check=True)
            for f in range(CH // FM):
                for k in (0, 1, 2):
                    for b in range(BPC):
                        w = k + FM * f
                        first = (k == 0) and not (warm and b == 0 and f == 0)
                        nc.tensor.matmul(
                            out=psd[:, b, FM * f:FM * (f + 1)],
                            lhsT=wk8_sb[:, :, k, :],
                            rhs=xt8[:, c, :, b, w:w + FM],
                            start=first, stop=(k == 2),
                            perf_mode=DR, skip_group_check=True)
            for f in range(CH // FM):
                for i, (h, k) in enumerate(
                        (h, k) for h in (0, 1) for k in (0, 1, 2)):
                    for b in range(BPC):
                        w = k + FM * f
                        nc.tensor.matmul(
                            out=pP[N * b:N * (b + 1), FM * f:FM * (f + 1)],
                            lhsT=wkB_sb[:, h, k, :],
                            rhs=xt[:, c, h, b, w:w + FM],
                            start=(i == 0), stop=(i == 5),
                            skip_group_check=True)

        def emit_tail_ops(c, psd, pP, s_prev, lo, hi, part):
            """softplus/Ad/u/bx/scan for chunk c, token cols [lo, hi)."""
            ad_sb = nsb.tile([P, CH], XDT, tag="ad")
            u_sb = nsb.tile([P, CH], XDT, tag="u")
            bx_sb = nsb.tile([P, CH], XDT, tag="bx")
            e_sb = nsb.tile([P, BPC, CH], XDT, tag="e", name=f"e_{c}{part}")
            l_sb = nsb.tile([P, BPC, CH], XDT, tag="l", name=f"l_{c}{part}")
            # exp split by f-half: psd's first half is released early,
            # unblocking the next chunk's f0 matmuls (psd bufs=1)
            mid = (lo + hi) // 2
            pieces = ((lo, mid), (mid, hi)) if hi - lo > FM else ((lo, hi),)
            for (a, b2) in pieces:
                nc.scalar.activation(
                    out=e_sb[:, :, a:b2], in_=psd[:, :, a:b2],
                    func=mybir.ActivationFunctionType.Exp,
                    scale=1.0 / S8,
                    bias=pcols_sb[:, 0:1])
            nc.scalar.activation(
                out=l_sb[:, :, lo:hi], in_=e_sb[:, :, lo:hi],
                func=mybir.ActivationFunctionType.Ln,
                bias=1.0)
            for b in range(BPC):
                nc.scalar.activation(
                    out=ad_sb[N * b:N * (b + 1), lo:hi],
                    in_=l_sb[N:P, b, lo:hi],
                    func=mybir.ActivationFunctionType.Exp,
                    scale=pcols_sb[N:P, 1:2])
            for b in range(BPC):
                nc.vector.scalar_tensor_tensor(
                    out=u_sb[N * b:N * (b + 1), lo:hi],
                    in0=pP[N * b:N * (b + 1), lo:hi],
                    scalar=pcols_sb[0:N, 2:3],
                    in1=l_sb[0:N, b, lo:hi],
                    op0=AOP.add, op1=AOP.mult)
            nc.vector.scalar_tensor_tensor(
                out=bx_sb[:, lo:hi], in0=ad_sb[:, lo:hi], scalar=-1.0,
                in1=u_sb[:, lo:hi], op0=AOP.add, op1=AOP.mult)
            s_tile = scanp.tile([P, CH], XDT, tag="s")
            nc.vector.tensor_tensor_scan(
                out=s_tile[:, lo:hi], data0=ad_sb[:, lo:hi],
                data1=bx_sb[:, lo:hi],
                initial=(0.0 if s_prev is None else s_prev),
                op0=AOP.mult, op1=AOP.add)
            return s_tile

        s_tile = None
        for c in range(NCH):
            psd = psum.tile([P, BPC, CH], FP, tag="sd", name=f"sd_{c}", bufs=1)
            pP = psum.tile([P, CH], FP, tag="bm", name=f"bm_{c}", bufs=2)
            emit_mm_block(c, psd, pP, warm=(c == 0))
            init = None if c == 0 else s_tile[:, CH - 1:CH]
            if c == NCH - 1:
                # split the final chunk's elementwise tail to shorten the
                # end-of-kernel serial chain
                s_half = emit_tail_ops(c, psd, pP, init, 0, FM, "a")
                s_tile = emit_tail_ops(c, psd, pP, s_half[:, FM - 1:FM],
                                       FM, CH, "b")
            else:
                s_tile = emit_tail_ops(c, psd, pP, init, 0, CH, "")

        # tail: y = s_last @ blockdiag(CmT*invA) + conv(x)[L-1] @ DmT + ybias
        # x[L-1] sits at local col CH+k-... global col L = (NCH-1)*CH + CH;
        # within the last chunk's window (starts at global (NCH-1)*CH) the
        # taps x[L-2], x[L-1] are local cols CH, CH+1... tap k local CH-1+k+1
        py = psum.tile([1, 2 * O], FP, tag="bm", bufs=2)
        for b in range(BPC):
            for h in (0, 1):
                for k in (0, 1):  # taps 0,1 of xc[L-1]; tap 2 is x[L] = 0
                    nc.tensor.matmul(
                        out=py[0:1, O * b:O * (b + 1)],
                        lhsT=xt[:, NCH - 1, h, b, CH - 1 + k:CH + k],
                        rhs=dmT_sb[:, h, k, :],
                        start=(b == 0 and h == 0 and k == 0), stop=False,
                        skip_group_check=True)
        nc.tensor.matmul(out=py, lhsT=s_tile[:, CH - 1:CH], rhs=cmT_sb,
                         start=False, stop=True, skip_group_check=True)
        y_sb = consts.tile([1, 2 * O], FP, tag="ysb")
        nc.vector.tensor_add(y_sb, py, ybias_sb)
        nc.sync.dma_start(out=y, in_=y_sb)

    nc.compile()
    return nc


def _to_np16(a):
    return np.asarray(a, np.float32).astype(np.float16)


def _prep_params(sel_W, sel_b, selection_bias, A_log, Bm, Cm, Dm,
                 delta_W, delta_b, conv_w, conv_b):
    f = np.float32
    sel_W = np.asarray(sel_W, f)
    delta_W = np.asarray(delta_W, f)
    Bm = np.asarray(Bm, f)
    Cm = np.asarray(Cm, f)
    Dm = np.asarray(Dm, f)
    conv_w = np.asarray(conv_w, f)      # [D, 1, 3]
    conv_b = np.asarray(conv_b, f)
    sel_b = np.asarray(sel_b, f)
    selection_bias = np.asarray(selection_bias, f)
    delta_b = np.asarray(delta_b, f)
    A_log = np.asarray(A_log, f)

    A = -np.exp(A_log.astype(np.float64))
    invA = 1.0 / (A + 1e-8)
    cw = conv_w[:, 0, :]                # [D, 3]

    Wsd = np.concatenate([sel_W, delta_W], axis=0)        # [128, D]
    wk8 = np.zeros((P, 2, 3, P), f)
    wkB = np.zeros((P, 2, 3, N), f)
    for h in (0, 1):
        for k in (0, 1, 2):
            Wf = Wsd * cw[None, :, k]
            wk8[:, h, k, :] = Wf[:, h * P:(h + 1) * P].T * S8
            Bf = Bm * cw[None, :, k]
            wkB[:, h, k, :] = Bf[:, h * P:(h + 1) * P].T

    pcols = np.zeros((P, 3), f)
    pcols[:, 0] = np.concatenate([sel_b + selection_bias + sel_W @ conv_b,
                                  delta_b + delta_W @ conv_b])
    pcols[:, 1] = np.tile(A.astype(f), 2)
    pcols[:, 2] = np.tile(Bm @ conv_b, 2)

    cmT = np.zeros((P, 2 * O), f)
    blk = (Cm.T.astype(np.float64) * invA[:, None]).astype(f)  # [N, O]
    cmT[0:N, 0:O] = blk
    cmT[N:2 * N, O:2 * O] = blk

    dmT = np.zeros((P, 2, 2, O), f)
    for h in (0, 1):
        for k in (0, 1):
            Df = Dm * cw[None, :, k]
            dmT[:, h, k, :] = Df[:, h * P:(h + 1) * P].T

    ybias = np.tile(Dm @ conv_b, 2)[None, :].astype(f)

    return dict(wk8=wk8.astype(ml_dtypes.float8_e4m3),
                wkB=_to_np16(wkB), pcols=pcols,
                cmT=_to_np16(cmT), dmT=_to_np16(dmT), ybias=ybias)


_CACHED = {}


def _get_program():
    if "nc" not in _CACHED:
        _CACHED["nc"] = _build_program()
    return _CACHED["nc"]


def kernel(x, sel_W, sel_b, selection_bias, A_log, Bm, Cm, Dm,
           delta_W, delta_b, conv_w, conv_b, _trace=False):
    x = np.asarray(x, np.float32)
    params = _prep_params(sel_W, sel_b, selection_bias, A_log, Bm, Cm,
                          Dm, delta_W, delta_b, conv_w, conv_b)
    # host-side transpose to [ncore, P, 2h, 2b, LW] fp16, zero pad cols,
    # then chunk into [ncore, P, NCH, 2, BPC, CW] windows (3-col halo)
    xT = x.transpose(0, 2, 1).reshape(NCORES, BPC, 2, P, L)
    pad = np.zeros((NCORES, P, 2, BPC, LW + 1), np.float16)
    pad[:, :, :, :, 1:L + 1] = xT.transpose(0, 3, 2, 1, 4)
    xt_full = np.zeros((NCORES, P, NCH, 2, BPC, CW), np.float16)
    for c in range(NCH):
        xt_full[:, :, c] = pad[:, :, :, :, c * CH:c * CH + CW]
    xt8_full = xt_full.astype(ml_dtypes.float8_e4m3)
    nc = _get_program()
    in_maps = []
    for c in range(NCORES):
        m = dict(params)
        m["xs"] = np.ascontiguousarray(xt_full[c])
        m["xs8"] = np.ascontiguousarray(xt8_full[c])
        in_maps.append(m)
    res = run_bass_kernel_spmd(nc, in_maps, core_ids=list(range(NCORES)),
                               trace=_trace)
    out = np.concatenate(
        [res.results[c]["y"].reshape(BPC, O) for c in range(NCORES)], axis=0)
    if _trace:
        _CACHED["last_results"] = res
    return out


# revision 14
# speedup vs baseline: 1.0885x; 1.0885x over previous
"""Trainium2 Bass kernel for EnhancedMambaStateSpace.

Full inputs in, full output out. Data-parallel over batch across 8 cores
(2 batch rows per core); SSM params replicated and pre-folded on host.

Math (per batch row b):
  xc = depthwise_conv1d(x, conv_w, pad=1) + conv_b
  sel = softplus(xc @ sel_W.T + sel_b + selection_bias)
  delta = softplus(xc @ delta_W.T + delta_b)
  A = -exp(A_log); Ad = exp(delta * A)
  Bx = (Ad - 1)/(A + 1e-8) * sel * (xc @ Bm.T)
  s_t = Ad_t * s_{t-1} + Bx_t  (scan over L, keep last)
  y = s_L @ Cm.T + xc[:, -1] @ Dm.T

Device layout: x is transposed and CHUNKED on host into
[P, NCH, 2h, 2b, CW] (CW = CH+3: each chunk carries its 3-col conv halo),
in both fp16 and fp8e4m3 — each chunk lands with ONE contiguous-per-
partition DMA (128 descriptor rows), keeping HW descriptor-gen off the
critical path. The conv is fully folded into the projection weights
(3 taps = 3 shifted matmul streams), so no explicit conv runs on any
engine. The sel|delta block runs in fp8 with DoubleRow perf mode: one
pass contracts both 128-halves of D (h is the DoubleRow k-tile dim); the
fp8 weights are pre-scaled by S8 on host and descaled for free via the
exp ACT's scale operand. The Bm block (numerically critical: its output
feeds y linearly) stays fp16. The recurrence is a DVE tensor_tensor_scan,
batch-packed [b0|b1] on 128 partitions. Warm-up dummy matmuls (discarded
via start=True PSUM reset) bring the PE out of its low p-state before
real data lands. The last chunk's elementwise tail runs in two
half-chunks to shorten the end-of-kernel serial chain.
"""

from contextlib import ExitStack

import ml_dtypes
import numpy as np

import concourse.bacc as bacc
import concourse.tile as tile
from concourse import mybir
from concourse.bass_utils import run_bass_kernel_spmd

B, L, D, N, O = 16, 4096, 256, 64, 256
P = 128          # partitions
CH = 1024        # tokens per chunk
NCH = L // CH    # 4 chunks
CW = CH + 3      # chunk window incl 3-col conv halo
BPC = 2          # batch rows per core
NCORES = 8
LW = L + 2       # pad col 0 (x[-1]=0) and col L+1 (x[L]=0)
FM = 512         # ISA max moving free dim
S8 = 256.0       # fp8 weight pre-scale (descaled in the exp ACT)
NWARM = 6        # PE p-state warm-up matmuls

FP = mybir.dt.float32
XDT = mybir.dt.float16
F8 = mybir.dt.float8e4
AOP = mybir.AluOpType
DR = mybir.MatmulPerfMode.DoubleRow

_ONE_TABLE = "natural_log_exp_and_others"


def _patch_act_tables():
    """Keep Exp/Ln/Copy resolvable only via one ACT table so the
    act-table-load pass never thrashes between tables (1283ns per load)."""
    import concourse.hw_specs as hw_specs
    import concourse.bacc as _bacc
    if getattr(_bacc, "_act_tables_patched", False):
        return
    orig = hw_specs.get_activation_tables

    def patched(module_arch):
        tabs = orig(module_arch)
        drop = {mybir.ActivationFunctionType.Exp,
                mybir.ActivationFunctionType.Ln,
                mybir.ActivationFunctionType.Copy}
        out = {}
        for name, funcs in tabs.items():
            if name == _ONE_TABLE:
                out[name] = funcs
            else:
                out[name] = funcs - drop
        return out

    _bacc.get_activation_tables = patched
    _bacc._act_tables_patched = True


def _build_program():
    _patch_act_tables()
    nc = bacc.Bacc("TRN2", target_bir_lowering=False, debug=False)

    # x chunked on host: xs[kd, c, h, b, w]; w = global col (1+t) - c*CH
    xs = nc.dram_tensor("xs", [P, NCH, 2, BPC, CW], XDT,
                        kind="ExternalInput").ap()
    xs8 = nc.dram_tensor("xs8", [P, NCH, 2, BPC, CW], F8,
                         kind="ExternalInput").ap()
    # fp8 tap-folded lhsT for sel|delta: wk8[kd, h, k, j] =
    #   S8 * Wsd[j, 128h+kd] * cw[128h+kd, k]; h is the DoubleRow k-tile dim
    wk8 = nc.dram_tensor("wk8", [P, 2, 3, P], F8, kind="ExternalInput").ap()
    # fp16 tap-folded lhsT for Bm: wkB[kd, h, k, n] = Bm[n, 128h+kd]*cw[.., k]
    wkB = nc.dram_tensor("wkB", [P, 2, 3, N], XDT, kind="ExternalInput").ap()
    # col 0: softplus bias (sel|delta), col 1: A tiled, col 2: Bm@conv_b tiled
    pcols = nc.dram_tensor("pcols", [P, 3], FP, kind="ExternalInput").ap()
    cmT = nc.dram_tensor("cmT", [P, 2 * O], XDT, kind="ExternalInput").ap()
    dmT = nc.dram_tensor("dmT", [P, 2, 2, O], XDT, kind="ExternalInput").ap()
    ybias = nc.dram_tensor("ybias", [1, 2 * O], FP, kind="ExternalInput").ap()
    y = nc.dram_tensor("y", [1, 2 * O], FP, kind="ExternalOutput").ap()

    with tile.TileContext(nc) as tc, ExitStack() as ctx:
        consts = ctx.enter_context(tc.tile_pool(name="consts", bufs=1))
        xtp = ctx.enter_context(tc.tile_pool(name="xtp", bufs=1))
        nsb = ctx.enter_context(tc.tile_pool(name="nsb", bufs=2))
        scanp = ctx.enter_context(tc.tile_pool(name="scanp", bufs=2))
        psum = ctx.enter_context(tc.tile_pool(name="psum", bufs=1, space="PSUM"))

        wk8_sb = consts.tile([P, 2, 3, P], F8, tag="wk8")
        # never-written-by-DMA garbage weights for the p-state warm-up
        dumw_sb = consts.tile([P, 2, P], F8, tag="dumw")
        wkB_sb = consts.tile([P, 2, 3, N], XDT, tag="wkB")
        pcols_sb = consts.tile([P, 3], FP, tag="pcols")
        cmT_sb = consts.tile([P, 2 * O], XDT, tag="cmT")
        dmT_sb = consts.tile([P, 2, 2, O], XDT, tag="dmT")
        ybias_sb = consts.tile([1, 2 * O], FP, tag="ybias")

        # whole-sequence chunked x, SBUF-resident, fp16 (Bm block) and
        # fp8 (sel|delta block); chunk-major so each chunk's DMA is one
        # contiguous run per partition
        xt = xtp.tile([P, NCH, 2, BPC, CW], XDT, tag="xt")
        xt8 = xtp.tile([P, NCH, 2, BPC, CW], F8, tag="xt8")

        # warm-up weights zeroed (values never consumed: the real psd
        # group's start=True resets PSUM); gpsimd is otherwise idle
        nc.gpsimd.memset(dumw_sb, 0.0)

        # sync carries the fp8 stream (psd block inputs), scalar carries
        # the fp16 stream (pP block inputs) + tail consts.
        nc.sync.dma_start(out=wk8_sb, in_=wk8)
        nc.sync.dma_start(out=xt8[:, 0], in_=xs8[:, 0])
        nc.sync.dma_start(out=pcols_sb, in_=pcols)
        # Scheduler guard: the compile-time list scheduler orders each
        # engine's stream by simulated readiness, and its DMA model can
        # (wrongly) decide the pP block's inputs land before the psd
        # block's, statically hoisting 5us of fp16 matmuls ahead of the
        # psd->softplus critical chain. Gate wkB behind the x8 chunk-0
        # arrival with real data deps: g1 reads xt8 chunk 0 (RAW on its
        # DMA), g2 reads g1 + the wkB tile (making the wkB DMA WAR-wait
        # on g2), so pP c0 can never be ready before psd c0.
        guard_sb = consts.tile([P, 1], FP, tag="guard")
        nc.scalar.activation(out=guard_sb, in_=xt8[:, 0, 0, 0, 0:1],
                             func=mybir.ActivationFunctionType.Copy)
        nc.vector.tensor_tensor(out=guard_sb, in0=guard_sb,
                                in1=wkB_sb[:, 0, 0, 0:1], op=AOP.add)
        nc.scalar.dma_start(out=wkB_sb, in_=wkB)
        nc.scalar.dma_start(out=xt[:, 0], in_=xs[:, 0])
        for c in range(1, NCH):
            nc.sync.dma_start(out=xt8[:, c], in_=xs8[:, c])
            nc.scalar.dma_start(out=xt[:, c], in_=xs[:, c])
        nc.scalar.dma_start(out=cmT_sb, in_=cmT)
        nc.scalar.dma_start(out=dmT_sb, in_=dmT)
        nc.scalar.dma_start(out=ybias_sb, in_=ybias)

        def emit_mm_block(c, psd, pP, warm=False):
            """Projection matmuls for chunk c: psd (fp8 DoubleRow, both
            halves contracted per pass) first to unblock the softplus
            chain, then the fp16 pP block. f-ordered so the f0 half only
            needs... both f come from the same chunk DMA now."""
            if warm:
                # p-state warm-up: matmuls on garbage inputs whose PSUM
                # result is discarded (the real group's start=True resets
                # the accumulator), so the PE exits its low p-state while
                # the first DMAs are still in flight. rhs reads the (not
                # yet DMA'd) last chunk of xt8, so the only input dep is
                # the tiny dumw memset.
                with tc.high_priority():
                    for i in range(NWARM):
                        nc.tensor.matmul(
                            out=psd[:, 0, 0:FM],
                            lhsT=dumw_sb,
                            rhs=xt8[:, NCH - 1, :, 1, 1 + i:1 + i + FM],
                            start=(i == 0), stop=(i == NWARM - 1),
                            perf_mode=DR, skip_group_check=True)
            for f in range(CH // FM):
                for k in (0, 1, 2):
                    for b in range(BPC):
                        w = k + FM * f
                        nc.tensor.matmul(
                            out=psd[:, b, FM * f:FM * (f + 1)],
                            lhsT=wk8_sb[:, :, k, :],
                            rhs=xt8[:, c, :, b, w:w + FM],
                            start=(k == 0), stop=(k == 2),
                            perf_mode=DR, skip_group_check=True)
            for f in range(CH // FM):
                for i, (h, k) in enumerate(
                        (h, k) for h in (0, 1) for k in (0, 1, 2)):
                    for b in range(BPC):
                        w = k + FM * f
                        nc.tensor.matmul(
                            out=pP[N * b:N * (b + 1), FM * f:FM * (f + 1)],
                            lhsT=wkB_sb[:, h, k, :],
                            rhs=xt[:, c, h, b, w:w + FM],
                            start=(i == 0), stop=(i == 5),
                            skip_group_check=True)

        def emit_tail_ops(c, psd, pP, s_prev, lo, hi, part):
            """softplus/Ad/u/bx/scan for chunk c, token cols [lo, hi)."""
            ad_sb = nsb.tile([P, CH], XDT, tag="ad")
            u_sb = nsb.tile([P, CH], XDT, tag="u")
            bx_sb = nsb.tile([P, CH], XDT, tag="bx")
            e_sb = nsb.tile([P, BPC, CH], XDT, tag="e", name=f"e_{c}{part}")
            l_sb = nsb.tile([P, BPC, CH], XDT, tag="l", name=f"l_{c}{part}")
            # exp split by f-half: psd's first half is released early,
            # unblocking the next chunk's f0 matmuls (psd bufs=1)
            mid = (lo + hi) // 2
            pieces = ((lo, mid), (mid, hi)) if hi - lo > FM else ((lo, hi),)
            for (a, b2) in pieces:
                nc.scalar.activation(
                    out=e_sb[:, :, a:b2], in_=psd[:, :, a:b2],
                    func=mybir.ActivationFunctionType.Exp,
                    scale=1.0 / S8,
                    bias=pcols_sb[:, 0:1])
            nc.scalar.activation(
                out=l_sb[:, :, lo:hi], in_=e_sb[:, :, lo:hi],
                func=mybir.ActivationFunctionType.Ln,
                bias=1.0)
            for b in range(BPC):
                nc.scalar.activation(
                    out=ad_sb[N * b:N * (b + 1), lo:hi],
                    in_=l_sb[N:P, b, lo:hi],
                    func=mybir.ActivationFunctionType.Exp,
                    scale=pcols_sb[N:P, 1:2])
            for b in range(BPC):
                nc.vector.scalar_tensor_tensor(
                    out=u_sb[N * b:N * (b + 1), lo:hi],
                    in0=pP[N * b:N * (b + 1), lo:hi],
                    scalar=pcols_sb[0:N, 2:3],
                    in1=l_sb[0:N, b, lo:hi],
                    op0=AOP.add, op1=AOP.mult)
            nc.vector.scalar_tensor_tensor(
                out=bx_sb[:, lo:hi], in0=ad_sb[:, lo:hi], scalar=-1.0,
                in1=u_sb[:, lo:hi], op0=AOP.add, op1=AOP.mult)
            s_tile = scanp.tile([P, CH], XDT, tag="s")
            nc.vector.tensor_tensor_scan(
                out=s_tile[:, lo:hi], data0=ad_sb[:, lo:hi],
                data1=bx_sb[:, lo:hi],
                initial=(0.0 if s_prev is None else s_prev),
                op0=AOP.mult, op1=AOP.add)
            return s_tile

        s_tile = None
        for c in range(NCH):
            psd = psum.tile([P, BPC, CH], FP, tag="sd", name=f"sd_{c}", bufs=1)
            pP = psum.tile([P, CH], FP, tag="bm", name=f"bm_{c}", bufs=2)
            emit_mm_block(c, psd, pP, warm=(c == 0))
            init = None if c == 0 else s_tile[:, CH - 1:CH]
            if c == NCH - 1:
                # split the final chunk's elementwise tail to shorten the
                # end-of-kernel serial chain
                s_half = emit_tail_ops(c, psd, pP, init, 0, FM, "a")
                s_tile = emit_tail_ops(c, psd, pP, s_half[:, FM - 1:FM],
                                       FM, CH, "b")
            else:
                s_tile = emit_tail_ops(c, psd, pP, init, 0, CH, "")

        # tail: y = s_last @ blockdiag(CmT*invA) + conv(x)[L-1] @ DmT + ybias
        # x[L-1] sits at local col CH+k-... global col L = (NCH-1)*CH + CH;
        # within the last chunk's window (starts at global (NCH-1)*CH) the
        # taps x[L-2], x[L-1] are local cols CH, CH+1... tap k local CH-1+k+1
        py = psum.tile([1, 2 * O], FP, tag="bm", bufs=2)
        for b in range(BPC):
            for h in (0, 1):
                for k in (0, 1):  # taps 0,1 of xc[L-1]; tap 2 is x[L] = 0
                    nc.tensor.matmul(
                        out=py[0:1, O * b:O * (b + 1)],
                        lhsT=xt[:, NCH - 1, h, b, CH - 1 + k:CH + k],
                        rhs=dmT_sb[:, h, k, :],
                        start=(b == 0 and h == 0 and k == 0), stop=False,
                        skip_group_check=True)
        nc.tensor.matmul(out=py, lhsT=s_tile[:, CH - 1:CH], rhs=cmT_sb,
                         start=False, stop=True, skip_group_check=True)
        y_sb = consts.tile([1, 2 * O], FP, tag="ysb")
        nc.vector.tensor_add(y_sb, py, ybias_sb)
        nc.sync.dma_start(out=y, in_=y_sb)

    nc.compile()
    return nc


def _to_np16(a):
    return np.asarray(a, np.float32).astype(np.float16)


def _prep_params(sel_W, sel_b, selection_bias, A_log, Bm, Cm, Dm,
                 delta_W, delta_b, conv_w, conv_b):
    f = np.float32
    sel_W = np.asarray(sel_W, f)
    delta_W = np.asarray(delta_W, f)
    Bm = np.asarray(Bm, f)
    Cm = np.asarray(Cm, f)
    Dm = np.asarray(Dm, f)
    conv_w = np.asarray(conv_w, f)      # [D, 1, 3]
    conv_b = np.asarray(conv_b, f)
    sel_b = np.asarray(sel_b, f)
    selection_bias = np.asarray(selection_bias, f)
    delta_b = np.asarray(delta_b, f)
    A_log = np.asarray(A_log, f)

    A = -np.exp(A_log.astype(np.float64))
    invA = 1.0 / (A + 1e-8)
    cw = conv_w[:, 0, :]                # [D, 3]

    Wsd = np.concatenate([sel_W, delta_W], axis=0)        # [128, D]
    wk8 = np.zeros((P, 2, 3, P), f)
    wkB = np.zeros((P, 2, 3, N), f)
    for h in (0, 1):
        for k in (0, 1, 2):
            Wf = Wsd * cw[None, :, k]
            wk8[:, h, k, :] = Wf[:, h * P:(h + 1) * P].T * S8
            Bf = Bm * cw[None, :, k]
            wkB[:, h, k, :] = Bf[:, h * P:(h + 1) * P].T

    pcols = np.zeros((P, 3), f)
    pcols[:, 0] = np.concatenate([sel_b + selection_bias + sel_W @ conv_b,
                                  delta_b + delta_W @ conv_b])
    pcols[:, 1] = np.tile(A.astype(f), 2)
    pcols[:, 2] = np.tile(Bm @ conv_b, 2)

    cmT = np.zeros((P, 2 * O), f)
    blk = (Cm.T.astype(np.float64) * invA[:, None]).astype(f)  # [N, O]
    cmT[0:N, 0:O] = blk
    cmT[N:2 * N, O:2 * O] = blk

    dmT = np.zeros((P, 2, 2, O), f)
    for h in (0, 1):
        for k in (0, 1):
            Df = Dm * cw[None, :, k]
            dmT[:, h, k, :] = Df[:, h * P:(h + 1) * P].T

    ybias = np.tile(Dm @ conv_b, 2)[None, :].astype(f)

    return dict(wk8=wk8.astype(ml_dtypes.float8_e4m3),
                wkB=_to_np16(wkB), pcols=pcols,
                cmT=_to_np16(cmT), dmT=_to_np16(dmT), ybias=ybias)


_CACHED = {}


def _get_program():
    if "nc" not in _CACHED:
        _CACHED["nc"] = _build_program()
    return _CACHED["nc"]


def kernel(x, sel_W, sel_b, selection_bias, A_log, Bm, Cm, Dm,
           delta_W, delta_b, conv_w, conv_b, _trace=False):
    x = np.asarray(x, np.float32)
    params = _prep_params(sel_W, sel_b, selection_bias, A_log, Bm, Cm,
                          Dm, delta_W, delta_b, conv_w, conv_b)
    # host-side transpose to [ncore, P, 2h, 2b, LW] fp16, zero pad cols,
    # then chunk into [ncore, P, NCH, 2, BPC, CW] windows (3-col halo)
    xT = x.transpose(0, 2, 1).reshape(NCORES, BPC, 2, P, L)
    pad = np.zeros((NCORES, P, 2, BPC, LW + 1), np.float16)
    pad[:, :, :, :, 1:L + 1] = xT.transpose(0, 3, 2, 1, 4)
    xt_full = np.zeros((NCORES, P, NCH, 2, BPC, CW), np.float16)
    for c in range(NCH):
        xt_full[:, :, c] = pad[:, :, :, :, c * CH:c * CH + CW]
    xt8_full = xt_full.astype(ml_dtypes.float8_e4m3)
    nc = _get_program()
    in_maps = []
    for c in range(NCORES):
        m = dict(params)
        m["xs"] = np.ascontiguousarray(xt_full[c])
        m["xs8"] = np.ascontiguousarray(xt8_full[c])
        in_maps.append(m)
    res = run_bass_kernel_spmd(nc, in_maps, core_ids=list(range(NCORES)),
                               trace=_trace)
    out = np.concatenate(
        [res.results[c]["y"].reshape(BPC, O) for c in range(NCORES)], axis=0)
    if _trace:
        _CACHED["last_results"] = res
    return out


# revision 18
# speedup vs baseline: 1.3088x; 1.2024x over previous
"""Trainium2 Bass kernel for EnhancedMambaStateSpace.

Full inputs in, full output out. Data-parallel over batch across 8 cores
(2 batch rows per core); SSM params replicated and pre-folded on host.

Math (per batch row b):
  xc = depthwise_conv1d(x, conv_w, pad=1) + conv_b
  sel = softplus(xc @ sel_W.T + sel_b + selection_bias)
  delta = softplus(xc @ delta_W.T + delta_b)
  A = -exp(A_log); Ad = exp(delta * A)
  Bx = (Ad - 1)/(A + 1e-8) * sel * (xc @ Bm.T)
  s_t = Ad_t * s_{t-1} + Bx_t  (scan over L, keep last)
  y = s_L @ Cm.T + xc[:, -1] @ Dm.T

Device layout: x is transposed and CHUNKED on host into
[P, NCH, 2h, 2b, CW] (CW = CH+3: each chunk carries its 3-col conv halo),
in both fp16 and fp8e4m3 — each chunk lands with ONE contiguous-per-
partition DMA (128 descriptor rows), keeping HW descriptor-gen off the
critical path. The conv is fully folded into the projection weights
(3 taps = 3 shifted matmul streams), so no explicit conv runs on any
engine. The sel|delta block runs in fp8 with DoubleRow perf mode: one
pass contracts both 128-halves of D (h is the DoubleRow k-tile dim); the
fp8 weights are pre-scaled by S8 on host and descaled for free via the
exp ACT's scale operand. The Bm block (numerically critical: its output
feeds y linearly) stays fp16. The recurrence is a DVE tensor_tensor_scan,
batch-packed [b0|b1] on 128 partitions. Warm-up dummy matmuls (discarded
via start=True PSUM reset) bring the PE out of its low p-state before
real data lands. The last chunk's elementwise tail runs in two
half-chunks to shorten the end-of-kernel serial chain.
"""

from contextlib import ExitStack

import ml_dtypes
import numpy as np

import concourse.bacc as bacc
import concourse.tile as tile
from concourse import mybir
from concourse.bass_utils import run_bass_kernel_spmd

B, L, D, N, O = 16, 4096, 256, 64, 256
P = 128          # partitions
CH = 1024        # tokens per chunk
NCH = L // CH    # 4 chunks
CW = CH + 3      # chunk window incl 3-col conv halo
BPC = 2          # batch rows per core
NCORES = 8
LW = L + 2       # pad col 0 (x[-1]=0) and col L+1 (x[L]=0)
FM = 512         # ISA max moving free dim
S8 = 256.0       # fp8 weight pre-scale (descaled in the exp ACT)
NWARM = 10       # PE p-state warm-up matmuls

FP = mybir.dt.float32
XDT = mybir.dt.float16
F8 = mybir.dt.float8e4
AOP = mybir.AluOpType
DR = mybir.MatmulPerfMode.DoubleRow

_ONE_TABLE = "natural_log_exp_and_others"


def _patch_act_tables():
    """Keep Exp/Ln/Copy resolvable only via one ACT table so the
    act-table-load pass never thrashes between tables (1283ns per load)."""
    import concourse.hw_specs as hw_specs
    import concourse.bacc as _bacc
    if getattr(_bacc, "_act_tables_patched", False):
        return
    orig = hw_specs.get_activation_tables

    def patched(module_arch):
        tabs = orig(module_arch)
        drop = {mybir.ActivationFunctionType.Exp,
                mybir.ActivationFunctionType.Ln,
                mybir.ActivationFunctionType.Copy}
        out = {}
        for name, funcs in tabs.items():
            if name == _ONE_TABLE:
                out[name] = funcs
            else:
                out[name] = funcs - drop
        return out

    _bacc.get_activation_tables = patched
    _bacc._act_tables_patched = True


def _build_program():
    _patch_act_tables()
    nc = bacc.Bacc("TRN2", target_bir_lowering=False, debug=False)

    # x chunked on host: xs[kd, c, h, b, w]; w = global col (1+t) - c*CH
    xs = nc.dram_tensor("xs", [P, NCH, 2, BPC, CW], XDT,
                        kind="ExternalInput").ap()
    xs8 = nc.dram_tensor("xs8", [P, NCH, 2, BPC, CW], F8,
                         kind="ExternalInput").ap()
    # fp8 tap-folded lhsT for sel|delta: wk8[kd, h, k, j] =
    #   S8 * Wsd[j, 128h+kd] * cw[128h+kd, k]; h is the DoubleRow k-tile dim
    wk8 = nc.dram_tensor("wk8", [P, 2, 3, P], F8, kind="ExternalInput").ap()
    # fp16 tap-folded lhsT for Bm: wkB[kd, h, k, n] = Bm[n, 128h+kd]*cw[.., k]
    wkB = nc.dram_tensor("wkB", [P, 2, 3, N], XDT, kind="ExternalInput").ap()
    # col 0: softplus bias (sel|delta), col 1: A tiled, col 2: Bm@conv_b tiled
    pcols = nc.dram_tensor("pcols", [P, 3], FP, kind="ExternalInput").ap()
    cmT = nc.dram_tensor("cmT", [P, 2 * O], XDT, kind="ExternalInput").ap()
    dmT = nc.dram_tensor("dmT", [P, 2, 2, O], XDT, kind="ExternalInput").ap()
    ybias = nc.dram_tensor("ybias", [1, 2 * O], FP, kind="ExternalInput").ap()
    y = nc.dram_tensor("y", [1, 2 * O], FP, kind="ExternalOutput").ap()

    with tile.TileContext(nc) as tc, ExitStack() as ctx:
        consts = ctx.enter_context(tc.tile_pool(name="consts", bufs=1))
        xtp = ctx.enter_context(tc.tile_pool(name="xtp", bufs=1))
        nsb = ctx.enter_context(tc.tile_pool(name="nsb", bufs=2))
        scanp = ctx.enter_context(tc.tile_pool(name="scanp", bufs=2))
        psum = ctx.enter_context(tc.tile_pool(name="psum", bufs=1, space="PSUM"))

        wk8_sb = consts.tile([P, 2, 3, P], F8, tag="wk8")
        # never-written-by-DMA garbage weights for the p-state warm-up
        dumw_sb = consts.tile([P, 2, P], F8, tag="dumw")
        wkB_sb = consts.tile([P, 2, 3, N], XDT, tag="wkB")
        pcols_sb = consts.tile([P, 3], FP, tag="pcols")
        cmT_sb = consts.tile([P, 2 * O], XDT, tag="cmT")
        dmT_sb = consts.tile([P, 2, 2, O], XDT, tag="dmT")
        ybias_sb = consts.tile([1, 2 * O], FP, tag="ybias")

        # whole-sequence chunked x, SBUF-resident, fp16 (Bm block) and
        # fp8 (sel|delta block); chunk-major so each chunk's DMA is one
        # contiguous run per partition
        xt = xtp.tile([P, NCH, 2, BPC, CW], XDT, tag="xt")
        xt8 = xtp.tile([P, NCH, 2, BPC, CW], F8, tag="xt8")

        # warm-up weights zeroed (values never consumed: the real psd
        # group's start=True resets PSUM); gpsimd is otherwise idle
        nc.gpsimd.memset(dumw_sb, 0.0)

        # p-state warm-up, emitted BEFORE any x DMA so its rhs read of
        # xt8's last chunk has no RAW dep (the later chunk DMA takes a
        # harmless WAR wait instead): the PE exits its low p-state while
        # the first DMAs are still in flight. PSUM garbage is discarded
        # when the real psd c0 group start=True resets the accumulator.
        psd0 = psum.tile([P, BPC, CH], FP, tag="sd", name="sd_0", bufs=1)
        with tc.high_priority():
            for i in range(NWARM):
                nc.tensor.matmul(
                    out=psd0[:, 0, 0:FM],
                    lhsT=dumw_sb,
                    rhs=xt8[:, NCH - 1, :, 1, 1 + i:1 + i + FM],
                    start=(i == 0), stop=(i == NWARM - 1),
                    perf_mode=DR, skip_group_check=True)

        # all x traffic rides the sync queue in just-in-time order (it
        # alone sustains the core's HBM share; a second big queue just
        # fights it for bandwidth); scalar carries only the small consts.
        nc.sync.dma_start(out=wk8_sb, in_=wk8)
        nc.sync.dma_start(out=xt8[:, 0], in_=xs8[:, 0])
        nc.sync.dma_start(out=pcols_sb, in_=pcols)
        nc.scalar.dma_start(out=cmT_sb, in_=cmT)
        nc.scalar.dma_start(out=dmT_sb, in_=dmT)
        nc.scalar.dma_start(out=ybias_sb, in_=ybias)
        # Scheduler guard: the compile-time list scheduler orders each
        # engine's stream by simulated readiness, and its DMA model can
        # (wrongly) decide the pP block's inputs land before the psd
        # block's, statically hoisting 5us of fp16 matmuls ahead of the
        # psd->softplus critical chain. Gate wkB behind the x8 chunk-0
        # arrival with real data deps: g1 reads xt8 chunk 0 (RAW on its
        # DMA), g2 reads g1 + the wkB tile (making the wkB DMA WAR-wait
        # on g2), so pP c0 can never be ready before psd c0.
        guard_sb = consts.tile([P, 1], FP, tag="guard")
        nc.scalar.activation(out=guard_sb, in_=xt8[:, 0, 0, 0, 0:1],
                             func=mybir.ActivationFunctionType.Copy)
        nc.vector.tensor_tensor(out=guard_sb, in0=guard_sb,
                                in1=wkB_sb[:, 0, 0, 0:1], op=AOP.add)
        nc.scalar.dma_start(out=wkB_sb, in_=wkB)
        nc.sync.dma_start(out=xt[:, 0], in_=xs[:, 0])
        for c in range(1, NCH):
            nc.sync.dma_start(out=xt8[:, c], in_=xs8[:, c])
            nc.sync.dma_start(out=xt[:, c], in_=xs[:, c])

        def emit_mm_block(c, psd, pP):
            """Projection matmuls for chunk c: psd (fp8 DoubleRow, both
            halves contracted per pass) first to unblock the softplus
            chain, then the fp16 pP block."""
            for f in range(CH // FM):
                for k in (0, 1, 2):
                    for b in range(BPC):
                        w = k + FM * f
                        nc.tensor.matmul(
                            out=psd[:, b, FM * f:FM * (f + 1)],
                            lhsT=wk8_sb[:, :, k, :],
                            rhs=xt8[:, c, :, b, w:w + FM],
                            start=(k == 0), stop=(k == 2),
                            perf_mode=DR, skip_group_check=True)
            for f in range(CH // FM):
                for i, (h, k) in enumerate(
                        (h, k) for h in (0, 1) for k in (0, 1, 2)):
                    for b in range(BPC):
                        w = k + FM * f
                        nc.tensor.matmul(
                            out=pP[N * b:N * (b + 1), FM * f:FM * (f + 1)],
                            lhsT=wkB_sb[:, h, k, :],
                            rhs=xt[:, c, h, b, w:w + FM],
                            start=(i == 0), stop=(i == 5),
                            skip_group_check=True)

        def emit_tail_ops(c, psd, pP, s_prev, lo, hi, part):
            """softplus/Ad/u/bx/scan for chunk c, token cols [lo, hi)."""
            ad_sb = nsb.tile([P, CH], XDT, tag="ad")
            u_sb = nsb.tile([P, CH], XDT, tag="u")
            bx_sb = nsb.tile([P, CH], XDT, tag="bx")
            e_sb = nsb.tile([P, BPC, CH], XDT, tag="e", name=f"e_{c}{part}")
            l_sb = nsb.tile([P, BPC, CH], XDT, tag="l", name=f"l_{c}{part}")
            # exp split by f-half: psd's first half is released early,
            # unblocking the next chunk's f0 matmuls (psd bufs=1)
            mid = (lo + hi) // 2
            pieces = ((lo, mid), (mid, hi)) if hi - lo > FM else ((lo, hi),)
            for (a, b2) in pieces:
                nc.scalar.activation(
                    out=e_sb[:, :, a:b2], in_=psd[:, :, a:b2],
                    func=mybir.ActivationFunctionType.Exp,
                    scale=1.0 / S8,
                    bias=pcols_sb[:, 0:1])
            nc.scalar.activation(
                out=l_sb[:, :, lo:hi], in_=e_sb[:, :, lo:hi],
                func=mybir.ActivationFunctionType.Ln,
                bias=1.0)
            for b in range(BPC):
                nc.scalar.activation(
                    out=ad_sb[N * b:N * (b + 1), lo:hi],
                    in_=l_sb[N:P, b, lo:hi],
                    func=mybir.ActivationFunctionType.Exp,
                    scale=pcols_sb[N:P, 1:2])
            for b in range(BPC):
                nc.vector.scalar_tensor_tensor(
                    out=u_sb[N * b:N * (b + 1), lo:hi],
                    in0=pP[N * b:N * (b + 1), lo:hi],
                    scalar=pcols_sb[0:N, 2:3],
                    in1=l_sb[0:N, b, lo:hi],
                    op0=AOP.add, op1=AOP.mult)
            nc.vector.scalar_tensor_tensor(
                out=bx_sb[:, lo:hi], in0=ad_sb[:, lo:hi], scalar=-1.0,
                in1=u_sb[:, lo:hi], op0=AOP.add, op1=AOP.mult)
            s_tile = scanp.tile([P, CH], XDT, tag="s")
            nc.vector.tensor_tensor_scan(
                out=s_tile[:, lo:hi], data0=ad_sb[:, lo:hi],
                data1=bx_sb[:, lo:hi],
                initial=(0.0 if s_prev is None else s_prev),
                op0=AOP.mult, op1=AOP.add)
            return s_tile

        s_tile = None
        for c in range(NCH):
            psd = psd0 if c == 0 else psum.tile(
                [P, BPC, CH], FP, tag="sd", name=f"sd_{c}", bufs=1)
            pP = psum.tile([P, CH], FP, tag="bm", name=f"bm_{c}", bufs=2)
            emit_mm_block(c, psd, pP)
            init = None if c == 0 else s_tile[:, CH - 1:CH]
            if c == NCH - 1:
                # split the final chunk's elementwise tail to shorten the
                # end-of-kernel serial chain
                s_half = emit_tail_ops(c, psd, pP, init, 0, FM, "a")
                s_tile = emit_tail_ops(c, psd, pP, s_half[:, FM - 1:FM],
                                       FM, CH, "b")
            else:
                s_tile = emit_tail_ops(c, psd, pP, init, 0, CH, "")

        # tail: y = s_last @ blockdiag(CmT*invA) + conv(x)[L-1] @ DmT + ybias
        # x[L-1] sits at local col CH+k-... global col L = (NCH-1)*CH + CH;
        # within the last chunk's window (starts at global (NCH-1)*CH) the
        # taps x[L-2], x[L-1] are local cols CH, CH+1... tap k local CH-1+k+1
        py = psum.tile([1, 2 * O], FP, tag="bm", bufs=2)
        for b in range(BPC):
            for h in (0, 1):
                for k in (0, 1):  # taps 0,1 of xc[L-1]; tap 2 is x[L] = 0
                    nc.tensor.matmul(
                        out=py[0:1, O * b:O * (b + 1)],
                        lhsT=xt[:, NCH - 1, h, b, CH - 1 + k:CH + k],
                        rhs=dmT_sb[:, h, k, :],
                        start=(b == 0 and h == 0 and k == 0), stop=False,
                        skip_group_check=True)
        nc.tensor.matmul(out=py, lhsT=s_tile[:, CH - 1:CH], rhs=cmT_sb,
                         start=False, stop=True, skip_group_check=True)
        y_sb = consts.tile([1, 2 * O], FP, tag="ysb")
        nc.vector.tensor_add(y_sb, py, ybias_sb)
        nc.sync.dma_start(out=y, in_=y_sb)

    nc.compile()
    return nc


def _to_np16(a):
    return np.asarray(a, np.float32).astype(np.float16)


def _prep_params(sel_W, sel_b, selection_bias, A_log, Bm, Cm, Dm,
                 delta_W, delta_b, conv_w, conv_b):
    f = np.float32
    sel_W = np.asarray(sel_W, f)
    delta_W = np.asarray(delta_W, f)
    Bm = np.asarray(Bm, f)
    Cm = np.asarray(Cm, f)
    Dm = np.asarray(Dm, f)
    conv_w = np.asarray(conv_w, f)      # [D, 1, 3]
    conv_b = np.asarray(conv_b, f)
    sel_b = np.asarray(sel_b, f)
    selection_bias = np.asarray(selection_bias, f)
    delta_b = np.asarray(delta_b, f)
    A_log = np.asarray(A_log, f)

    A = -np.exp(A_log.astype(np.float64))
    invA = 1.0 / (A + 1e-8)
    cw = conv_w[:, 0, :]                # [D, 3]

    Wsd = np.concatenate([sel_W, delta_W], axis=0)        # [128, D]
    wk8 = np.zeros((P, 2, 3, P), f)
    wkB = np.zeros((P, 2, 3, N), f)
    for h in (0, 1):
        for k in (0, 1, 2):
            Wf = Wsd * cw[None, :, k]
            wk8[:, h, k, :] = Wf[:, h * P:(h + 1) * P].T * S8
            Bf = Bm * cw[None, :, k]
            wkB[:, h, k, :] = Bf[:, h * P:(h + 1) * P].T

    pcols = np.zeros((P, 3), f)
    pcols[:, 0] = np.concatenate([sel_b + selection_bias + sel_W @ conv_b,
                                  delta_b + delta_W @ conv_b])
    pcols[:, 1] = np.tile(A.astype(f), 2)
    pcols[:, 2] = np.tile(Bm @ conv_b, 2)

    cmT = np.zeros((P, 2 * O), f)
    blk = (Cm.T.astype(np.float64) * invA[:, None]).astype(f)  # [N, O]
    cmT[0:N, 0:O] = blk
    cmT[N:2 * N, O:2 * O] = blk

    dmT = np.zeros((P, 2, 2, O), f)
    for h in (0, 1):
        for k in (0, 1):
            Df = Dm * cw[None, :, k]
            dmT[:, h, k, :] = Df[:, h * P:(h + 1) * P].T

    ybias = np.tile(Dm @ conv_b, 2)[None, :].astype(f)

    return dict(wk8=wk8.astype(ml_dtypes.float8_e4m3),
                wkB=_to_np16(wkB), pcols=pcols,
                cmT=_to_np16(cmT), dmT=_to_np16(dmT), ybias=ybias)


_CACHED = {}


def _get_program():
    if "nc" not in _CACHED:
        _CACHED["nc"] = _build_program()
    return _CACHED["nc"]


def kernel(x, sel_W, sel_b, selection_bias, A_log, Bm, Cm, Dm,
           delta_W, delta_b, conv_w, conv_b, _trace=False):
    x = np.asarray(x, np.float32)
    params = _prep_params(sel_W, sel_b, selection_bias, A_log, Bm, Cm,
                          Dm, delta_W, delta_b, conv_w, conv_b)
    # host-side transpose to [ncore, P, 2h, 2b, LW] fp16, zero pad cols,
    # then chunk into [ncore, P, NCH, 2, BPC, CW] windows (3-col halo)
    xT = x.transpose(0, 2, 1).reshape(NCORES, BPC, 2, P, L)
    pad = np.zeros((NCORES, P, 2, BPC, LW + 1), np.float16)
    pad[:, :, :, :, 1:L + 1] = xT.transpose(0, 3, 2, 1, 4)
    xt_full = np.zeros((NCORES, P, NCH, 2, BPC, CW), np.float16)
    for c in range(NCH):
        xt_full[:, :, c] = pad[:, :, :, :, c * CH:c * CH + CW]
    xt8_full = xt_full.astype(ml_dtypes.float8_e4m3)
    nc = _get_program()
    in_maps = []
    for c in range(NCORES):
        m = dict(params)
        m["xs"] = np.ascontiguousarray(xt_full[c])
        m["xs8"] = np.ascontiguousarray(xt8_full[c])
        in_maps.append(m)
    res = run_bass_kernel_spmd(nc, in_maps, core_ids=list(range(NCORES)),
                               trace=_trace)
    out = np.concatenate(
        [res.results[c]["y"].reshape(BPC, O) for c in range(NCORES)], axis=0)
    if _trace:
        _CACHED["last_results"] = res
    return out
